# revision 1
# baseline (speedup 1.0000x reference)
"""GCNBlock (GCNConv + LayerNorm + LeakyReLU + residual) on 8 TRN2 NeuronCores.

Strategy (graph/data parallel over destination nodes):
  * 128-node output "windows" are assigned to cores (greedy-balanced).
  * Host computes degrees / edge norms, appends self-loops as ordinary
    edges, buckets edges by (core, src-half, window) and pads each bucket
    to whole 128-edge tiles.  All feature compute happens on device.
  * Device: dma_gather pulls x[src] rows from HBM; DVE builds a selection
    matrix S[e, j] = (dst_rel[e] == j) * w[e] in one fused op; the PE
    accumulates aggT[c, j] += Xg[e, c]^T @ S[e, j] per window in PSUM.
    Epilogue per window: (aggT)^T @ W + b, LayerNorm, LeakyReLU, + x.
  * Linearity trick: segment_sum(norm * x[src]) @ W == reference's
    segment_sum(norm * (xW)[src]) -- so xW is never materialized.

kernel(**inputs) takes the FULL inputs and returns the FULL [N, C] output.
"""

import math

import numpy as np

N = 50000
E = 600000
C = 128
P = 128
NCORES = 8
HALF = 25000  # int16 gather indices: split x into two row-halves
NWIN = (N + P - 1) // P  # 391 global windows
SLOTS = (NWIN + NCORES - 1) // NCORES  # 49 window slots per core
LN_EPS = 1e-5
ALPHA = 0.01
CH_TILES = 8  # tiles (of 128 edges) per dma_gather chunk (HW: <=1024 idxs/gather)

_CACHE: dict = {}
LAST_RESULT = None


# --------------------------------------------------------------------------
# Host-side sharding / index prep
# --------------------------------------------------------------------------
def _host_prep(x, edge_index):
    src = np.asarray(edge_index[0], dtype=np.int64)
    dst = np.asarray(edge_index[1], dtype=np.int64)

    deg = (np.bincount(dst, minlength=N) + 1.0).astype(np.float64)
    dinv = 1.0 / np.sqrt(deg)

    nodes = np.arange(N, dtype=np.int64)
    asrc = np.concatenate([src, nodes])
    adst = np.concatenate([dst, nodes])
    aw = np.concatenate(
        [(dinv[src] * dinv[dst]).astype(np.float32), (dinv * dinv).astype(np.float32)]
    )

    win = adst >> 7
    half = (asrc >= HALF).astype(np.int64)

    # per-window edge counts, split by source half
    cnt = np.zeros((NWIN, 2), np.int64)
    np.add.at(cnt, (win, half), 1)
    tot = cnt.sum(axis=1)

    # greedy balanced assignment of windows to cores (largest first)
    order = np.argsort(-tot, kind="stable")
    loads = np.zeros(NCORES, np.int64)
    nwins = np.zeros(NCORES, np.int64)
    core_of_win = np.full(NWIN, -1, np.int64)
    for w in order:
        cand = np.where(nwins < SLOTS)[0]
        c = cand[np.argmin(loads[cand])]
        core_of_win[w] = c
        loads[c] += tot[w]
        nwins[c] += 1

    # slot assignment: windows within a core sorted by size desc so the
    # per-slot max-over-cores caps stay tight
    slot_wins = np.full((NCORES, SLOTS), -1, np.int64)
    slot_of_win = np.zeros(NWIN, np.int64)
    for c in range(NCORES):
        ws = sorted(np.where(core_of_win == c)[0], key=lambda w: -tot[w])
        for j, w in enumerate(ws):
            slot_wins[c, j] = w
            slot_of_win[w] = j

    # per (slot, half) capacity in tiles (shared across cores)
    cap = np.zeros((SLOTS, 2), np.int64)
    for j in range(SLOTS):
        for h in (0, 1):
            m = 0
            for c in range(NCORES):
                w = slot_wins[c, j]
                if w >= 0:
                    m = max(m, cnt[w, h])
            cap[j, h] = (m + P - 1) // P
    t_lo = int(cap[:, 0].sum())
    t_hi = int(cap[:, 1].sum())
    t_total = t_lo + t_hi

    tile_off = np.zeros((SLOTS, 2), np.int64)
    tile_off[:, 0] = np.cumsum(cap[:, 0]) - cap[:, 0]
    tile_off[:, 1] = t_lo + np.cumsum(cap[:, 1]) - cap[:, 1]

    # destination slot (flat edge position) for every augmented edge
    ecore = core_of_win[win]
    eslot = slot_of_win[win]
    key = (ecore * 2 + half) * SLOTS + eslot
    sidx = np.argsort(key, kind="stable")
    key_s = key[sidx]
    uniq, start = np.unique(key_s, return_index=True)
    within = np.arange(key_s.size, dtype=np.int64) - start[
        np.searchsorted(uniq, key_s)
    ]

    base_by_key = np.zeros(NCORES * 2 * SLOTS, np.int64)
    for c in range(NCORES):
        for h in (0, 1):
            for j in range(SLOTS):
                base_by_key[(c * 2 + h) * SLOTS + j] = tile_off[j, h] * P
    dest = base_by_key[key_s] + within

    nslots = t_total * P
    gidx = np.zeros((NCORES, nslots), np.int16)
    wgt = np.zeros((NCORES, nslots), np.float32)
    drel = np.zeros((NCORES, nslots), np.float32)
    core_s = key_s // (2 * SLOTS)
    gidx[core_s, dest] = (asrc[sidx] % HALF).astype(np.int16)
    wgt[core_s, dest] = aw[sidx]
    drel[core_s, dest] = (adst[sidx] & 127).astype(np.float32)

    # dma_gather index layout: element i -> [i % 16, i // 16], the 16-row
    # block replicated across all 128 partitions (8 gpsimd cores)
    g16 = gidx.reshape(NCORES, nslots // 16, 16).transpose(0, 2, 1)
    gidx_w = np.ascontiguousarray(np.tile(g16, (1, 8, 1)))  # [NCORES, 128, T*8]
    drel_t = np.ascontiguousarray(
        drel.reshape(NCORES, t_total, P).transpose(0, 2, 1)
    )  # [NCORES, 128, T]
    wgt_t = np.ascontiguousarray(wgt.reshape(NCORES, t_total, P).transpose(0, 2, 1))

    # residual rows per (core, slot)
    xpad = np.zeros((NWIN * P, C), np.float32)
    xpad[:N] = x
    xwin = np.zeros((NCORES, SLOTS * P, C), np.float32)
    for c in range(NCORES):
        for j in range(SLOTS):
            w = slot_wins[c, j]
            if w >= 0:
                xwin[c, j * P : (j + 1) * P] = xpad[w * P : (w + 1) * P]

    return dict(
        cap=cap,
        t_lo=t_lo,
        t_hi=t_hi,
        t_total=t_total,
        slot_wins=slot_wins,
        gidx_w=gidx_w,
        drel_t=drel_t,
        wgt_t=wgt_t,
        xwin=xwin,
    )


# --------------------------------------------------------------------------
# Device program
# --------------------------------------------------------------------------
def _build_program(cap, t_lo, t_hi, trivial_affine, variant=()):
    """variant: experiment flags for timing surgery (sim only):
    'noS'    -- skip per-tile S build (use one const S tile)
    'nomm'   -- skip aggregation matmuls
    'nogath' -- skip dma_gather calls
    'noepi'  -- skip per-window epilogues (just copy psum out)
    """
    variant = frozenset(variant)
    from contextlib import ExitStack

    import concourse.bass as bass  # noqa: F401
    import concourse.mybir as mybir
    import concourse.tile as tile
    from concourse import bacc

    f32 = mybir.dt.float32
    i16 = mybir.dt.int16
    Alu = mybir.AluOpType
    Act = mybir.ActivationFunctionType
    Ax = mybir.AxisListType

    t_total = t_lo + t_hi

    nc = bacc.Bacc(
        "TRN2",
        target_bir_lowering=False,
        debug=False,
        num_devices=NCORES,
        num_swdge_queues=4,
    )

    x_d = nc.dram_tensor("x", [N, C], f32, kind="ExternalInput")
    xw_d = nc.dram_tensor("xwin", [SLOTS * P, C], f32, kind="ExternalInput")
    gi_d = nc.dram_tensor("gidx", [P, t_total * 8], i16, kind="ExternalInput")
    dr_d = nc.dram_tensor("drel", [P, t_total], f32, kind="ExternalInput")
    wg_d = nc.dram_tensor("wgt", [P, t_total], f32, kind="ExternalInput")
    w_d = nc.dram_tensor("w", [C, C], f32, kind="ExternalInput")
    bb_d = nc.dram_tensor("bb", [P, C], f32, kind="ExternalInput")
    io_d = nc.dram_tensor("iota", [P, P], f32, kind="ExternalInput")
    if not trivial_affine:
        gm_d = nc.dram_tensor("gmb", [P, C], f32, kind="ExternalInput")
        bt_d = nc.dram_tensor("btb", [P, C], f32, kind="ExternalInput")
    out_d = nc.dram_tensor("out", [SLOTS * P, C], f32, kind="ExternalOutput")

    x_ap = x_d.ap()
    src_views = [x_ap[0:HALF, :], x_ap[HALF:N, :]]

    with tile.TileContext(nc) as tc, ExitStack() as ctx:
        const = ctx.enter_context(tc.tile_pool(name="const", bufs=1))
        W_t = const.tile([C, C], f32)
        nc.sync.dma_start(W_t[:], w_d.ap())
        bb_t = const.tile([P, C], f32)
        nc.sync.dma_start(bb_t[:], bb_d.ap())
        io_t = const.tile([P, P], f32)
        nc.sync.dma_start(io_t[:], io_d.ap())
        if not trivial_affine:
            gm_t = const.tile([P, C], f32)
            nc.sync.dma_start(gm_t[:], gm_d.ap())
            bt_t = const.tile([P, C], f32)
            nc.sync.dma_start(bt_t[:], bt_d.ap())
        eps_t = const.tile([P, 1], f32)
        nc.gpsimd.memset(eps_t[:], LN_EPS)
        gi_t = const.tile([P, t_total * 8], i16)
        nc.sync.dma_start(gi_t[:], gi_d.ap())
        dr_t = const.tile([P, t_total], f32)
        nc.sync.dma_start(dr_t[:], dr_d.ap())
        wg_t = const.tile([P, t_total], f32)
        nc.sync.dma_start(wg_t[:], wg_d.ap())
        part_t = const.tile([P, SLOTS * P], f32)

        gpool = ctx.enter_context(tc.tile_pool(name="gath", bufs=3))
        spool = ctx.enter_context(tc.tile_pool(name="sel", bufs=6))
        psumA = ctx.enter_context(tc.tile_pool(name="psA", bufs=3, space="PSUM"))
        psumB = ctx.enter_context(tc.tile_pool(name="psB", bufs=2, space="PSUM"))
        wpool = ctx.enter_context(tc.tile_pool(name="xw", bufs=3))
        epool = ctx.enter_context(tc.tile_pool(name="ep", bufs=3))
        stat = ctx.enter_context(tc.tile_pool(name="stat", bufs=6))

        qn = [0]

        S_const = None
        if "noS" in variant:
            S_const = const.tile([P, P], f32)
            nc.gpsimd.memset(S_const[:], 0.0)

        def epilogue(j, pj_hi, has_lo):
            if "noepi" in variant:
                o0 = epool.tile([P, C], f32, tag="o")
                src0 = pj_hi[:] if pj_hi is not None else part_t[:, j * P : (j + 1) * P]
                nc.scalar.activation(o0[:], src0, Act.Copy, bias=0.0, scale=1.0)
                nc.sync.dma_start(out_d.ap()[j * P : (j + 1) * P, :], o0[:])
                return
            jcols = slice(j * P, (j + 1) * P)
            aggT = epool.tile([P, C], f32, tag="aggT")
            if pj_hi is not None and has_lo:
                nc.vector.tensor_tensor(
                    out=aggT[:], in0=pj_hi[:], in1=part_t[:, jcols], op=Alu.add
                )
            elif pj_hi is not None:
                nc.vector.tensor_copy(out=aggT[:], in_=pj_hi[:])
            else:
                nc.vector.tensor_copy(out=aggT[:], in_=part_t[:, jcols])
            ps2 = psumB.tile([P, C], f32, tag="ps2")
            nc.tensor.matmul(ps2[:], lhsT=aggT[:], rhs=W_t[:], start=True, stop=True)

            t_sb = epool.tile([P, C], f32, tag="tsb")
            nc.vector.tensor_tensor(out=t_sb[:], in0=ps2[:], in1=bb_t[:], op=Alu.add)
            sum1 = stat.tile([P, 1], f32, tag="sum")
            nc.vector.tensor_reduce(
                out=sum1[:], in_=t_sb[:], axis=Ax.X, op=Alu.add
            )
            mu = stat.tile([P, 1], f32, tag="mu")
            nc.vector.tensor_scalar(
                out=mu[:], in0=sum1[:], scalar1=1.0 / C, scalar2=None, op0=Alu.mult
            )
            cen = epool.tile([P, C], f32, tag="cen")
            nc.vector.tensor_scalar(
                out=cen[:], in0=t_sb[:], scalar1=mu[:, 0:1], scalar2=None,
                op0=Alu.subtract,
            )
            sq = epool.tile([P, C], f32, tag="sq")
            ssq = stat.tile([P, 1], f32, tag="var")
            nc.scalar.activation(sq[:], cen[:], Act.Square, accum_out=ssq[:])
            stdt = stat.tile([P, 1], f32, tag="std")
            nc.scalar.activation(
                stdt[:], ssq[:], Act.Sqrt, bias=eps_t[:, 0:1], scale=1.0 / C
            )
            rstd = stat.tile([P, 1], f32, tag="rstd")
            nc.vector.reciprocal(rstd[:], stdt[:])
            yn = epool.tile([P, C], f32, tag="yn")
            nc.vector.tensor_scalar(
                out=yn[:], in0=cen[:], scalar1=rstd[:, 0:1], scalar2=None, op0=Alu.mult
            )
            if not trivial_affine:
                y2 = epool.tile([P, C], f32, tag="y2")
                nc.vector.tensor_tensor(out=y2[:], in0=yn[:], in1=gm_t[:], op=Alu.mult)
                yn = epool.tile([P, C], f32, tag="y3")
                nc.vector.tensor_tensor(out=yn[:], in0=y2[:], in1=bt_t[:], op=Alu.add)
            sc = epool.tile([P, C], f32, tag="sc")
            nc.scalar.activation(sc[:], yn[:], Act.Copy, bias=0.0, scale=ALPHA)
            lr = epool.tile([P, C], f32, tag="lr")
            nc.vector.tensor_tensor(out=lr[:], in0=yn[:], in1=sc[:], op=Alu.max)
            xw_t = wpool.tile([P, C], f32, tag="xw")
            nc.sync.dma_start(xw_t[:], xw_d.ap()[j * P : (j + 1) * P, :])
            o = epool.tile([P, C], f32, tag="o")
            nc.vector.tensor_tensor(out=o[:], in0=lr[:], in1=xw_t[:], op=Alu.add)
            nc.sync.dma_start(out_d.ap()[j * P : (j + 1) * P, :], o[:])

        for h in (0, 1):
            region_base = 0 if h == 0 else t_lo
            tiles = []  # (slot, first, last)
            for j in range(SLOTS):
                nt = int(cap[j, h])
                for k in range(nt):
                    tiles.append((j, k == 0, k == nt - 1))
            cur = {}
            for c0 in range(0, len(tiles), CH_TILES):
                chunk = tiles[c0 : c0 + CH_TILES]
                n = len(chunk)
                t0 = region_base + c0
                xg = gpool.tile([P, CH_TILES, P], f32, tag="xg")
                if "nogath" not in variant:
                    nc.gpsimd.dma_gather(
                        xg[:, :n, :],
                        src_views[h],
                        gi_t[:, t0 * 8 : (t0 + n) * 8],
                        num_idxs=n * P,
                        num_idxs_reg=n * P,
                        elem_size=C,
                        elem_step=C,
                        queue_num=qn[0],
                    )
                    qn[0] = (qn[0] + 1) % 4
                for i, (j, first, last) in enumerate(chunk):
                    t = t0 + i
                    if "noS" in variant:
                        S = S_const
                    else:
                        S = spool.tile([P, P], f32, tag="S")
                        nc.vector.tensor_scalar(
                            out=S[:],
                            in0=io_t[:],
                            scalar1=dr_t[:, t : t + 1],
                            scalar2=wg_t[:, t : t + 1],
                            op0=Alu.is_equal,
                            op1=Alu.mult,
                        )
                    if first:
                        cur[j] = psumA.tile([P, P], f32, tag="agg", name=f"agg{h}_{j}")
                    if "nomm" not in variant:
                        nc.tensor.matmul(
                            cur[j][:], lhsT=xg[:, i, :], rhs=S[:], start=first,
                            stop=last,
                        )
                    if last:
                        pj = cur.pop(j)
                        if h == 0:
                            nc.scalar.activation(
                                part_t[:, j * P : (j + 1) * P],
                                pj[:],
                                Act.Copy,
                                bias=0.0,
                                scale=1.0,
                            )
                        else:
                            epilogue(j, pj, has_lo=cap[j, 0] > 0)
        # slots with hi-half empty
        for j in range(SLOTS):
            if cap[j, 1] == 0:
                epilogue(j, None, has_lo=cap[j, 0] > 0)

    nc.compile()
    return nc


# --------------------------------------------------------------------------
# Entry point
# --------------------------------------------------------------------------
def kernel(x, edge_index, W, b, gamma, beta):
    x = np.ascontiguousarray(np.asarray(x, dtype=np.float32))
    W = np.ascontiguousarray(np.asarray(W, dtype=np.float32))
    b = np.asarray(b, dtype=np.float32)
    gamma = np.asarray(gamma, dtype=np.float32)
    beta = np.asarray(beta, dtype=np.float32)

    prep = _host_prep(x, edge_index)
    cap = prep["cap"]
    trivial_affine = bool(np.all(gamma == 1.0) and np.all(beta == 0.0))

    key = (tuple(cap.flatten().tolist()), trivial_affine)
    if key not in _CACHE:
        _CACHE.clear()
        _CACHE[key] = _build_program(cap, prep["t_lo"], prep["t_hi"], trivial_affine)
    nc = _CACHE[key]

    iota = np.tile(np.arange(P, dtype=np.float32), (P, 1))
    bb = np.tile(b[None, :], (P, 1)).astype(np.float32)
    in_maps = []
    for c in range(NCORES):
        m = {
            "x": x,
            "xwin": prep["xwin"][c],
            "gidx": prep["gidx_w"][c],
            "drel": prep["drel_t"][c],
            "wgt": prep["wgt_t"][c],
            "w": W,
            "bb": bb,
            "iota": iota,
        }
        if not trivial_affine:
            m["gmb"] = np.tile(gamma[None, :], (P, 1)).astype(np.float32)
            m["btb"] = np.tile(beta[None, :], (P, 1)).astype(np.float32)
        in_maps.append(m)

    from concourse import bass_utils

    trace = bool(int(__import__("os").environ.get("BASS_TRACE", "0") or "0"))
    res = bass_utils.run_bass_kernel_spmd(
        nc,
        in_maps,
        core_ids=list(range(NCORES)),
        trace=trace,
        trace_cores=list(range(NCORES)) if trace else None,
    )
    global LAST_RESULT
    LAST_RESULT = res

    out = np.zeros((N, C), dtype=np.float32)
    slot_wins = prep["slot_wins"]
    for c in range(NCORES):
        oc = res.results[c]["out"]
        for j in range(SLOTS):
            w = slot_wins[c, j]
            if w < 0:
                continue
            r0 = w * P
            r1 = min(r0 + P, N)
            out[r0:r1] = oc[j * P : j * P + (r1 - r0)]
    return out



# revision 9
# speedup vs baseline: 1.5719x; 1.5719x over previous
"""GCNBlock (GCNConv + LayerNorm + LeakyReLU + residual) on 8 TRN2 NeuronCores.

Strategy (graph/data parallel over destination nodes), v2:
  * Degree normalization is folded into the node features on the host:
    xs = x * dinv (bf16).  Every edge message then has unit weight and the
    remaining dinv_dst factor is applied per destination row after the
    W matmul.  Self-loops are not materialized as edges; each window's own
    xs rows enter the aggregation via one identity-matmul (PE transpose).
  * 128-node output windows assigned to cores (greedy-balanced).  Edges
    bucketed per (core, slot, src-half); per-(slot,half) capacities are
    shared across cores (SPMD); per-core shortfall is padded with idx=-1
    at the group tail, which dma_gather trims for free.
  * Device: one dma_gather per (slot, half) chunk pulls xs[src] rows (bf16,
    256B descriptors) from HBM; DVE builds a pure one-hot S per tile; the
    PE accumulates aggT[c, j] += xs_g[e, c]^T @ S[e, j] in PSUM, seeded by
    aggT = xswin^T via the identity matmul.  Epilogue per window:
    (aggT)^T @ W (bf16), * dinv_dst + b, LayerNorm, LeakyReLU, + x.

kernel(**inputs) takes the FULL inputs and returns the FULL [N, C] output.
"""

import numpy as np

N = 50000
E = 600000
C = 128
P = 128
NCORES = 8
HALF = 25000  # int16 gather indices: split xs into two row-halves
NWIN = (N + P - 1) // P  # 391 global windows
SLOTS = (NWIN + NCORES - 1) // NCORES  # 49 window slots per core
LN_EPS = 1e-5
ALPHA = 0.01
CH_TILES = 8  # max tiles (of 128 edges) per dma_gather (HW: <=1024 idxs)
PAD_DREL = 200.0  # one-hot miss value for padded slots (exact in bf16)

_CACHE: dict = {}
LAST_RESULT = None


def _bf16(a):
    import ml_dtypes

    return np.ascontiguousarray(a.astype(ml_dtypes.bfloat16))


# --------------------------------------------------------------------------
# Host-side sharding / index prep
# --------------------------------------------------------------------------
def _host_prep(x, edge_index):
    src = np.asarray(edge_index[0], dtype=np.int64)
    dst = np.asarray(edge_index[1], dtype=np.int64)

    deg = (np.bincount(dst, minlength=N) + 1.0).astype(np.float64)
    dinv = (1.0 / np.sqrt(deg)).astype(np.float32)

    win = dst >> 7
    half = (src >= HALF).astype(np.int64)

    # per-window edge counts, split by source half
    cnt = np.zeros((NWIN, 2), np.int64)
    np.add.at(cnt, (win, half), 1)
    tot = cnt.sum(axis=1)

    # greedy balanced assignment of windows to cores (largest first)
    order = np.argsort(-tot, kind="stable")
    loads = np.zeros(NCORES, np.int64)
    nwins = np.zeros(NCORES, np.int64)
    core_of_win = np.full(NWIN, -1, np.int64)
    for w in order:
        cand = np.where(nwins < SLOTS)[0]
        c = cand[np.argmin(loads[cand])]
        core_of_win[w] = c
        loads[c] += tot[w]
        nwins[c] += 1

    # slot assignment: windows within a core sorted by size desc so the
    # per-slot max-over-cores caps stay tight
    slot_wins = np.full((NCORES, SLOTS), -1, np.int64)
    slot_of_win = np.zeros(NWIN, np.int64)
    for c in range(NCORES):
        ws = sorted(np.where(core_of_win == c)[0], key=lambda w: -tot[w])
        for j, w in enumerate(ws):
            slot_wins[c, j] = w
            slot_of_win[w] = j

    # per (slot, half) capacity in tiles (shared across cores)
    cap = np.zeros((SLOTS, 2), np.int64)
    for j in range(SLOTS):
        for h in (0, 1):
            m = 0
            for c in range(NCORES):
                w = slot_wins[c, j]
                if w >= 0:
                    m = max(m, cnt[w, h])
            cap[j, h] = (m + P - 1) // P
    t_total = int(cap.sum())

    # tile offsets: groups laid out in (slot, half) order
    tile_off = np.zeros((SLOTS, 2), np.int64)
    acc = 0
    for j in range(SLOTS):
        for h in (0, 1):
            tile_off[j, h] = acc
            acc += cap[j, h]

    # destination slot (flat position) for every edge
    ecore = core_of_win[win]
    eslot = slot_of_win[win]
    key = (ecore * SLOTS + eslot) * 2 + half
    sidx = np.argsort(key, kind="stable")
    key_s = key[sidx]
    uniq, start = np.unique(key_s, return_index=True)
    within = np.arange(key_s.size, dtype=np.int64) - start[
        np.searchsorted(uniq, key_s)
    ]

    base_by_key = np.zeros(NCORES * SLOTS * 2, np.int64)
    for c in range(NCORES):
        for j in range(SLOTS):
            for h in (0, 1):
                base_by_key[(c * SLOTS + j) * 2 + h] = tile_off[j, h] * P
    dest = base_by_key[key_s] + within

    nslots = t_total * P
    pad_idx = 0 if __import__("os").environ.get("BASS_PAD0") else -1
    gidx = np.full((NCORES, nslots), pad_idx, np.int16)  # -1 = trimmed tail pad
    drel = np.full((NCORES, nslots), PAD_DREL, np.float32)
    safe_trim = not __import__("os").environ.get("BASS_PAD0")
    core_s = key_s // (2 * SLOTS)
    gidx[core_s, dest] = (src[sidx] % HALF).astype(np.int16)
    drel[core_s, dest] = (dst[sidx] & 127).astype(np.float32)

    if safe_trim:
        # Per gather chunk keep a multiple-of-128 (>=128) prefix of valid
        # indices (pad with 0); only whole trailing tiles stay -1 (trimmed).
        cnt_cjh = np.zeros((NCORES, SLOTS, 2), np.int64)
        np.add.at(cnt_cjh, (ecore, eslot, half), 1)
        for c in range(NCORES):
            for j in range(SLOTS):
                for h in (0, 1):
                    nt = int(cap[j, h])
                    real_g = int(cnt_cjh[c, j, h])
                    for c0 in range(0, nt, 8):
                        n = min(8, nt - c0)
                        base = (tile_off[j, h] + c0) * P
                        real = min(max(real_g - c0 * P, 0), n * P)
                        keep = min(max(-(-real // P) * P, P), n * P)
                        if real < keep:
                            gidx[c, base + real : base + keep] = 0

    # dma_gather index layout: element i -> [i % 16, i // 16], the 16-row
    # block replicated across all 128 partitions (8 gpsimd cores)
    g16 = gidx.reshape(NCORES, nslots // 16, 16).transpose(0, 2, 1)
    gidx_w = np.ascontiguousarray(np.tile(g16, (1, 8, 1)))  # [NCORES, 128, T*8]
    drel_t = np.ascontiguousarray(
        drel.reshape(NCORES, t_total, P).transpose(0, 2, 1)
    )  # [NCORES, 128, T]

    # per-core window-ordered node data
    xpad = np.zeros((NWIN * P, C), np.float32)
    xpad[:N] = x
    dpad = np.ones(NWIN * P, np.float32)
    dpad[:N] = dinv
    xwin = np.zeros((NCORES, SLOTS * P, C), np.float32)  # residual rows
    xswin = np.zeros((NCORES, SLOTS * P, C), np.float32)  # xs rows (self loop)
    dwin = np.ones((NCORES, SLOTS * P), np.float32)  # dinv rows
    for c in range(NCORES):
        for j in range(SLOTS):
            w = slot_wins[c, j]
            if w >= 0:
                rows = slice(w * P, (w + 1) * P)
                xwin[c, j * P : (j + 1) * P] = xpad[rows]
                xswin[c, j * P : (j + 1) * P] = (
                    xpad[rows] * dpad[rows][:, None]
                )
                dwin[c, j * P : (j + 1) * P] = dpad[rows]

    xs = (x * dinv[:, None]).astype(np.float32)  # gather table (bf16 later)

    return dict(
        cap=cap,
        tile_off=tile_off,
        t_total=t_total,
        slot_wins=slot_wins,
        gidx_w=gidx_w,
        drel_t=drel_t,
        xwin=xwin,
        xswin=xswin,
        dwin=dwin,
        xs=xs,
    )


# --------------------------------------------------------------------------
# Device program
# --------------------------------------------------------------------------
def _build_program(cap, tile_off, t_total, trivial_affine):
    from contextlib import ExitStack

    import concourse.bass as bass  # noqa: F401
    import concourse.mybir as mybir
    import concourse.tile as tile
    from concourse import bacc

    f32 = mybir.dt.float32
    bf16 = mybir.dt.bfloat16
    i16 = mybir.dt.int16
    Alu = mybir.AluOpType
    Act = mybir.ActivationFunctionType
    Ax = mybir.AxisListType

    nc = bacc.Bacc(
        "TRN2",
        target_bir_lowering=False,
        debug=False,
        num_devices=NCORES,
        num_swdge_queues=4,
    )

    xs_d = nc.dram_tensor("xs", [N, C], bf16, kind="ExternalInput")
    xw_d = nc.dram_tensor("xwin", [P, SLOTS * C], f32, kind="ExternalInput")
    xsw_d = nc.dram_tensor("xswin", [P, SLOTS * C], bf16, kind="ExternalInput")
    dw_d = nc.dram_tensor("dwin", [P, SLOTS], f32, kind="ExternalInput")
    gi_d = nc.dram_tensor("gidx", [P, t_total * 8], i16, kind="ExternalInput")
    dr_d = nc.dram_tensor("drel", [P, t_total], f32, kind="ExternalInput")
    w_d = nc.dram_tensor("w", [C, C], bf16, kind="ExternalInput")
    bb_d = nc.dram_tensor("bb", [P, C], f32, kind="ExternalInput")
    io_d = nc.dram_tensor("iota", [P, P], bf16, kind="ExternalInput")
    id_d = nc.dram_tensor("ident", [P, P], bf16, kind="ExternalInput")
    if not trivial_affine:
        gm_d = nc.dram_tensor("gmb", [P, C], f32, kind="ExternalInput")
        bt_d = nc.dram_tensor("btb", [P, C], f32, kind="ExternalInput")
    out_d = nc.dram_tensor("out", [SLOTS * P, C], f32, kind="ExternalOutput")

    xs_ap = xs_d.ap()
    src_views = [xs_ap[0:HALF, :], xs_ap[HALF:N, :]]

    with tile.TileContext(nc) as tc, ExitStack() as ctx:
        const = ctx.enter_context(tc.tile_pool(name="const", bufs=1))
        W_t = const.tile([C, C], bf16)
        nc.sync.dma_start(W_t[:], w_d.ap())
        bb_t = const.tile([P, C], f32)
        nc.sync.dma_start(bb_t[:], bb_d.ap())
        io_t = const.tile([P, P], bf16)
        nc.sync.dma_start(io_t[:], io_d.ap())
        id_t = const.tile([P, P], bf16)
        nc.sync.dma_start(id_t[:], id_d.ap())
        if not trivial_affine:
            gm_t = const.tile([P, C], f32)
            nc.sync.dma_start(gm_t[:], gm_d.ap())
            bt_t = const.tile([P, C], f32)
            nc.sync.dma_start(bt_t[:], bt_d.ap())
        eps_t = const.tile([P, 1], f32)
        nc.gpsimd.memset(eps_t[:], LN_EPS)
        gi_t = const.tile([P, t_total * 8], i16)
        nc.sync.dma_start(gi_t[:], gi_d.ap())
        dr_t = const.tile([P, t_total], f32)
        nc.sync.dma_start(dr_t[:], dr_d.ap())
        dw_t = const.tile([P, SLOTS], f32)
        nc.sync.dma_start(dw_t[:], dw_d.ap())
        xw_t = const.tile([P, SLOTS * C], f32)
        nc.sync.dma_start(xw_t[:], xw_d.ap())
        xsw_t = const.tile([P, SLOTS * C], bf16)
        nc.sync.dma_start(xsw_t[:], xsw_d.ap())

        gpool = ctx.enter_context(tc.tile_pool(name="gath", bufs=4))
        spool = ctx.enter_context(tc.tile_pool(name="sel", bufs=6))
        psumA = ctx.enter_context(tc.tile_pool(name="psA", bufs=3, space="PSUM"))
        psumB = ctx.enter_context(tc.tile_pool(name="psB", bufs=2, space="PSUM"))
        epool = ctx.enter_context(tc.tile_pool(name="ep", bufs=3))
        stat = ctx.enter_context(tc.tile_pool(name="stat", bufs=6))

        qn = [0]

        # dma_gather trims trailing idx=-1 pads, leaving those xg slots
        # unwritten; zero the pool buffers once so stale SBUF is never NaN.
        for _ in range(4):
            xg0 = gpool.tile([P, CH_TILES, C], bf16, tag="xg")
            nc.vector.memset(xg0[:], 0.0)

        def epilogue(j, pj):
            jcols = slice(j * C, (j + 1) * C)
            aggT = epool.tile([P, C], bf16, tag="aggT")
            nc.vector.tensor_copy(out=aggT[:], in_=pj[:])
            ps2 = psumB.tile([P, C], f32, tag="ps2")
            nc.tensor.matmul(ps2[:], lhsT=aggT[:], rhs=W_t[:], start=True, stop=True)

            # * dinv_dst (scalar engine) then + b
            t1 = epool.tile([P, C], f32, tag="t1")
            nc.scalar.activation(t1[:], ps2[:], Act.Copy, bias=0.0,
                                 scale=dw_t[:, j : j + 1])
            t_sb = epool.tile([P, C], f32, tag="tsb")
            nc.vector.tensor_tensor(out=t_sb[:], in0=t1[:], in1=bb_t[:], op=Alu.add)

            sum1 = stat.tile([P, 1], f32, tag="sum")
            nc.vector.tensor_reduce(out=sum1[:], in_=t_sb[:], axis=Ax.X, op=Alu.add)
            mu = stat.tile([P, 1], f32, tag="mu")
            nc.vector.tensor_scalar(
                out=mu[:], in0=sum1[:], scalar1=1.0 / C, scalar2=None, op0=Alu.mult
            )
            cen = epool.tile([P, C], f32, tag="cen")
            nc.vector.tensor_scalar(
                out=cen[:], in0=t_sb[:], scalar1=mu[:, 0:1], scalar2=None,
                op0=Alu.subtract,
            )
            sq = epool.tile([P, C], f32, tag="sq")
            ssq = stat.tile([P, 1], f32, tag="var")
            nc.scalar.activation(sq[:], cen[:], Act.Square, accum_out=ssq[:])
            stdt = stat.tile([P, 1], f32, tag="std")
            nc.scalar.activation(
                stdt[:], ssq[:], Act.Sqrt, bias=eps_t[:, 0:1], scale=1.0 / C
            )
            rstd = stat.tile([P, 1], f32, tag="rstd")
            nc.vector.reciprocal(rstd[:], stdt[:])
            yn = epool.tile([P, C], f32, tag="yn")
            nc.vector.tensor_scalar(
                out=yn[:], in0=cen[:], scalar1=rstd[:, 0:1], scalar2=None, op0=Alu.mult
            )
            if not trivial_affine:
                y2 = epool.tile([P, C], f32, tag="y2")
                nc.vector.tensor_tensor(out=y2[:], in0=yn[:], in1=gm_t[:], op=Alu.mult)
                yn = epool.tile([P, C], f32, tag="y3")
                nc.vector.tensor_tensor(out=yn[:], in0=y2[:], in1=bt_t[:], op=Alu.add)
            sc = epool.tile([P, C], f32, tag="sc")
            nc.scalar.activation(sc[:], yn[:], Act.Copy, bias=0.0, scale=ALPHA)
            lr = epool.tile([P, C], f32, tag="lr")
            nc.vector.tensor_tensor(out=lr[:], in0=yn[:], in1=sc[:], op=Alu.max)
            o = epool.tile([P, C], f32, tag="o")
            nc.vector.tensor_tensor(out=o[:], in0=lr[:], in1=xw_t[:, jcols],
                                    op=Alu.add)
            nc.sync.dma_start(out_d.ap()[j * P : (j + 1) * P, :], o[:])

        for j in range(SLOTS):
            jcols = slice(j * C, (j + 1) * C)
            # seed aggT with the window's own xs rows (self-loop term)
            pj = psumA.tile([P, P], f32, tag="agg", name=f"agg{j}")
            only_seed = cap[j, 0] == 0 and cap[j, 1] == 0
            nc.tensor.matmul(
                pj[:], lhsT=xsw_t[:, jcols], rhs=id_t[:],
                start=True, stop=bool(only_seed),
            )
            for h in (0, 1):
                nt = int(cap[j, h])
                if nt == 0:
                    continue
                t0 = int(tile_off[j, h])
                last_half = h == 1 or cap[j, 1] == 0
                for c0 in range(0, nt, CH_TILES):
                    n = min(CH_TILES, nt - c0)
                    tc0 = t0 + c0
                    xg = gpool.tile([P, CH_TILES, C], bf16, tag="xg")
                    nc.gpsimd.dma_gather(
                        xg[:, :n, :],
                        src_views[h],
                        gi_t[:, tc0 * 8 : (tc0 + n) * 8],
                        num_idxs=n * P,
                        num_idxs_reg=n * P,
                        elem_size=C,
                        elem_step=C,
                        queue_num=qn[0],
                    )
                    qn[0] = (qn[0] + 1) % 4
                    for i in range(n):
                        t = tc0 + i
                        S = spool.tile([P, P], bf16, tag="S")
                        nc.vector.tensor_scalar(
                            out=S[:], in0=io_t[:],
                            scalar1=dr_t[:, t : t + 1], scalar2=None,
                            op0=Alu.is_equal,
                        )
                        last = last_half and (c0 + i == nt - 1)
                        nc.tensor.matmul(
                            pj[:], lhsT=xg[:, i, :], rhs=S[:],
                            start=False, stop=bool(last),
                        )
            epilogue(j, pj)

    nc.compile()
    return nc


# --------------------------------------------------------------------------
# Entry point
# --------------------------------------------------------------------------
def kernel(x, edge_index, W, b, gamma, beta):
    x = np.ascontiguousarray(np.asarray(x, dtype=np.float32))
    W = np.ascontiguousarray(np.asarray(W, dtype=np.float32))
    b = np.asarray(b, dtype=np.float32)
    gamma = np.asarray(gamma, dtype=np.float32)
    beta = np.asarray(beta, dtype=np.float32)

    prep = _host_prep(x, edge_index)
    cap = prep["cap"]
    trivial_affine = bool(np.all(gamma == 1.0) and np.all(beta == 0.0))

    key = (tuple(cap.flatten().tolist()), trivial_affine)
    if key not in _CACHE:
        _CACHE.clear()
        _CACHE[key] = _build_program(cap, prep["tile_off"], prep["t_total"],
                                     trivial_affine)
    nc = _CACHE[key]

    iota = _bf16(np.tile(np.arange(P, dtype=np.float32), (P, 1)))
    ident = _bf16(np.eye(P, dtype=np.float32))
    bb = np.tile(b[None, :], (P, 1)).astype(np.float32)
    xs16 = _bf16(prep["xs"])
    in_maps = []
    for c in range(NCORES):
        m = {
            "xs": xs16,
            "xwin": np.ascontiguousarray(
                prep["xwin"][c].reshape(SLOTS, P, C).transpose(1, 0, 2)
                .reshape(P, SLOTS * C)
            ),
            "xswin": _bf16(
                prep["xswin"][c].reshape(SLOTS, P, C).transpose(1, 0, 2)
                .reshape(P, SLOTS * C)
            ),
            "dwin": np.ascontiguousarray(
                prep["dwin"][c].reshape(SLOTS, P).T
            ),
            "gidx": prep["gidx_w"][c],
            "drel": prep["drel_t"][c],
            "w": _bf16(W),
            "bb": bb,
            "iota": iota,
            "ident": ident,
        }
        if not trivial_affine:
            m["gmb"] = np.tile(gamma[None, :], (P, 1)).astype(np.float32)
            m["btb"] = np.tile(beta[None, :], (P, 1)).astype(np.float32)
        in_maps.append(m)

    from concourse import bass_utils

    trace = bool(int(__import__("os").environ.get("BASS_TRACE", "0") or "0"))
    res = bass_utils.run_bass_kernel_spmd(
        nc,
        in_maps,
        core_ids=list(range(NCORES)),
        trace=trace,
        trace_cores=list(range(NCORES)) if trace else None,
    )
    global LAST_RESULT
    LAST_RESULT = res

    out = np.zeros((N, C), dtype=np.float32)
    slot_wins = prep["slot_wins"]
    for c in range(NCORES):
        oc = res.results[c]["out"]
        for j in range(SLOTS):
            w = slot_wins[c, j]
            if w < 0:
                continue
            r0 = w * P
            r1 = min(r0 + P, N)
            out[r0:r1] = oc[j * P : j * P + (r1 - r0)]
    return out
